# revision 35
# baseline (speedup 1.0000x reference)
"""Multi-head attention (with the reference's double-split_proj quirk) on 8
Trainium2 NeuronCores via Bass/Tile.

Sharding: core c handles batch b = c//4 and heads {4g..4g+3} where g = c%4
(data parallel on B, tensor parallel on heads). The double application of
_split_proj in the reference means the Q used for head i at attention row
j = (S/16)*h' + jj is q_proj[b, 16*jj + i, 64*h' + d]; per head that is a
gather of rows s ≡ i (mod 16) of q_proj over all 16 channel groups, so each
core only needs the 1/4 of query rows with s mod 16 in its head range —
sliced on the host. Everything runs transposed (channels on partitions,
positions on free): scores^T = K^T-chunks @ Q^T, softmax statistics come
from an appended ones-column on V (row 64 of the P@V output is the rowsum
Z), masking is a post-exp multiply by (1-mask)^T in bf16, and the output
projection consumes an AllGather of per-core attention features over each
batch's 4 cores.

v2 layout: all per-core device inputs travel as TWO dram tensors — one bf16
blob [128, TOTW] holding qts|kT|vT|mask|wq|wk|wv|wo pre-blocked on the host
into [128 partitions, ci, w] bands, and one tiny f32 blob for the biases —
so the whole input side is ~14 large DMAs instead of ~250 small ones
(per-DMA issue overhead on the DGE queues was a first-order cost), and the
per-dispatch PJRT argument marshaling drops from 13 buffers to 2.

Schedule: K projection and the first half of Q projection run up front
(psB "sc" psum ring); the remaining Q chunks and V-projection pairs
stream through the same transient ring inside the first attention piece
so PE never blocks on the slower input DMA. Attention runs head-major in
[SCW]-wide query pieces; each piece accumulates P@V over the 16 key
chunks in its own pv-ring slot, then normalizes with a DVE reciprocal on
the Z row followed by a rank-1 PE matmul (ones ⊗ 1/Z) that broadcasts
the reciprocal across partitions into the just-evacuated PV psum rows
(no DRAM round-trip, no cross-partition ops). Features AllGather
per-head quarter so three of the four gathers complete while later heads
are still in attention; the final head's quarter is the only gather in
the output-projection tail, and the earlier quarters' feature tiles are
pre-loaded so the tail's matmuls start immediately.

Precision: projected Q^T/K^T stay float32r (full-rate fp32) so the scores
matmul keeps fp32-class precision; P@V runs bf16 (mask multiply on VectorE
in bf16 2x mode); all accumulation fp32 in PSUM.
"""

import sys

for _p in ("/opt/trn_rl_repo",):
    if _p not in sys.path:
        sys.path.append(_p)

import numpy as np
import ml_dtypes

import concourse.bass as bass
import concourse.bacc as bacc
import concourse.mybir as mybir
import concourse.tile as tile
from concourse.bass_utils import run_bass_kernel_spmd

B = 2
D = 1024
H = 16
DH = 64
NCORES = 8
S_FULL = 2048

f32 = mybir.dt.float32
f32r = mybir.dt.float32r
bf16 = mybir.dt.bfloat16

_MODULES = {}


def _offsets(S):
    """Column offsets (elements) of each band in the bf16 blob."""
    SQ = S // 4
    off = {}
    cur = 0
    for name, w in [
        ("qts", 8 * SQ), ("kt", 8 * S), ("vt", 8 * S), ("mask", 16 * S),
        ("wq", 8 * D), ("wk", 8 * 256), ("wv", 8 * 256), ("wo", 8 * 256),
    ]:
        off[name] = cur
        cur += w
    return off, cur


def build_module(S, stub_collective=False):
    """Build + compile the per-core Bass module (same program on all cores).

    stub_collective=True replaces the AllGather with local DMAs (wrong
    results on cores > 0) so the module is single-core analyzable.
    """
    JJ = S // 16          # rows per (head, channel-group)
    SQ = 4 * JJ           # host-gathered query rows per core (4 heads)
    KC = S // 128         # number of 128-wide key chunks
    SCW = 1024 if S >= 2048 else S   # query-piece width
    NPC = S // SCW        # pieces per head
    NQ = 512              # f32r moving-operand chunk
    OFF, TOTW = _offsets(S)

    nc = bacc.Bacc("TRN2", target_bir_lowering=False, debug=False,
                   num_devices=NCORES)

    xb_d = nc.dram_tensor("xb", [128, TOTW], bf16, kind="ExternalInput")
    xf_d = nc.dram_tensor("xf", [128, 400], f32, kind="ExternalInput")
    out_d = nc.dram_tensor("ofinal", [256, S], f32, kind="ExternalOutput")

    def band(name, ci):
        w = {"qts": SQ, "kt": S, "vt": S, "mask": S,
             "wq": D, "wk": 256, "wv": 256, "wo": 256}[name]
        n = 16 if name == "mask" else 8
        v = xb_d[:, OFF[name]:OFF[name] + n * w].rearrange(
            "p (c w) -> p c w", c=n)
        return v if ci is None else v[:, ci, :]

    Exp = mybir.ActivationFunctionType.Exp
    Ident = mybir.ActivationFunctionType.Identity

    with tile.TileContext(nc) as tc:
        with (
            tc.tile_pool(name="persist", bufs=1) as pp,
            tc.tile_pool(name="stream", bufs=1) as sp,
            tc.tile_pool(name="dram", bufs=1, space="DRAM") as dp,
            tc.tile_pool(name="psB", bufs=1, space="PSUM") as psB,
            tc.tile_pool(name="psC", bufs=1, space="PSUM") as psC,
        ):
            # ------------- resident inputs, ordered by criticality -------------
            # scalar queue: wk -> qts -> wq halves (Q/K projection inputs)
            # sync queue:   kt0 -> bias -> kt1-3 -> mask quarters
            # gpsimd queue: wv -> vt pairs (in v_proj_pair) -> wo
            wk_sb = pp.tile([128, 8, 256], bf16, tag="wk")
            nc.scalar.dma_start(wk_sb[:], band("wk", None))
            qts_sb = pp.tile([128, 8, SQ], bf16, tag="qts")
            nc.scalar.dma_start(qts_sb[:], band("qts", None))
            wq_sb = pp.tile([128, 8, D], bf16, tag="wq")
            for wh in range(2):
                nc.scalar.dma_start(
                    wq_sb[:, :, 512 * wh:512 * (wh + 1)],
                    band("wq", None)[:, :, 512 * wh:512 * (wh + 1)])
            wv_sb = pp.tile([128, 8, 256], bf16, tag="wv")
            nc.gpsimd.dma_start(wv_sb[:], band("wv", None))

            bias_sb = pp.tile([128, 16], f32, tag="bias")
            bvrow_sb = pp.tile([1, 256], f32r, tag="bvrow")
            onesrow_sb = pp.tile([1, 128], f32r, tag="onesrow")

            mask_sb = pp.tile([128, KC, S], bf16, tag="mask")
            wo_sb = pp.tile([128, 8, 256], bf16, tag="wo")

            # per-head Q^T and K^T packed in pairs: head hi lives on
            # partitions 64*(hi%2) .. +64 of pair tile hi//2
            QTp = [pp.tile([128, S], f32r, tag=f"QTp{h}", name=f"QTp{h}")
                   for h in range(2)]
            KTp = [pp.tile([128, S], f32r, tag=f"KTp{h}", name=f"KTp{h}")
                   for h in range(2)]

            def QTs(hi):
                return QTp[hi // 2][64 * (hi % 2):64 * (hi % 2) + 64, :]

            def KTs(hi):
                return KTp[hi // 2][64 * (hi % 2):64 * (hi % 2) + 64, :]

            # V+bias per key chunk, augmented with a ones column per head:
            # cols 65*hi+d (d<64), ones at 65*hi+64
            VA = [pp.tile([128, 260], bf16, tag=f"VA{sc}", name=f"VA{sc}")
                  for sc in range(KC)]
            for sc in range(KC):
                nc.vector.memset(
                    VA[sc].rearrange("p (h x) -> p h x", h=4)[:, :, 64:65], 1.0)

            # per-(head, piece) gather buffers: the gather granule is one
            # piece, so only the very last piece's gather sits in the tail
            feats_dram = {(x, pc): dp.tile([64, SCW], bf16,
                                           tag=f"feats_dram{x}_{pc}",
                                           name=f"feats_dram{x}_{pc}")
                          for x in range(4) for pc in range(NPC)}
            featsall = {(x, pc): dp.tile([256, SCW], bf16,
                                         tag=f"featsall{x}_{pc}",
                                         name=f"featsall{x}_{pc}")
                        for x in range(4) for pc in range(NPC)}

            # ---------------- K projection ----------------
            # kt ring: [128, 8, 512] chunks of kT (8 contraction blocks x
            # 512 key positions)
            for nb in range(S // NQ):
                kt_t = sp.tile([128, 8, NQ], bf16, tag="kt", bufs=2,
                               name=f"kt{nb}")
                nc.sync.dma_start(
                    kt_t[:], band("kt", None)[:, :, NQ * nb:NQ * (nb + 1)])
                if nb == 0:
                    nc.sync.dma_start(bias_sb[:], xf_d[:, 0:16])
                    nc.sync.dma_start(bvrow_sb[:],
                                      xf_d[0:1, 16:272].bitcast(f32r))
                    nc.sync.dma_start(onesrow_sb[:],
                                      xf_d[0:1, 272:400].bitcast(f32r))
                k_ps = psB.tile([128, 2 * NQ], f32, tag="sc", bufs=2,
                                name=f"kps{nb}")
                for p in range(2):
                    for ci in range(8):
                        nc.tensor.matmul(
                            k_ps[:, NQ * p:NQ * (p + 1)],
                            wk_sb[:, ci, 128 * p:128 * (p + 1)],
                            kt_t[:, ci, :],
                            start=(ci == 0), stop=(ci == 7))
                for p in range(2):
                    for half in range(2):
                        hi = 2 * p + half
                        nc.vector.tensor_scalar_add(
                            KTs(hi)[:, NQ * nb:NQ * (nb + 1)],
                            k_ps[64 * half:64 * half + 64, NQ * p:NQ * (p + 1)],
                            bias_sb[64 * half:64 * half + 64, 8 + p:9 + p])

            # ---------------- Q projection (half now, half mid-attention) ---
            def q_proj_g4(g4):
                q_ps = psB.tile([128, 2 * NQ], f32, tag="sc", bufs=2,
                                name=f"qps{g4}")
                for j in range(2):
                    po = 2 * g4 + j
                    for ci in range(8):
                        nc.tensor.matmul(
                            q_ps[:, NQ * j:NQ * (j + 1)],
                            wq_sb[:, ci, 128 * po:128 * (po + 1)],
                            qts_sb[:, ci, :],
                            start=(ci == 0), stop=(ci == 7))
                for j in range(2):
                    po = 2 * g4 + j
                    for hi in range(4):
                        for half in range(2):
                            h2 = 2 * po + half
                            nc.vector.tensor_scalar_add(
                                QTs(hi)[:, JJ * h2:JJ * (h2 + 1)],
                                q_ps[64 * half:64 * half + 64,
                                     NQ * j + JJ * hi:NQ * j + JJ * (hi + 1)],
                                bias_sb[64 * half:64 * half + 64, po:po + 1])

            # chunks p0-3 feed the first attention pieces (query cols 0:1024)
            q_proj_g4(0)
            q_proj_g4(1)

            # ---------------- V projection ----------------
            # psum [128, 1024] with two 256-wide slices at 512 spacing so the
            # concurrent accumulation groups sit in different PSUM banks.
            # Pairs 0-1 land before attention needs VA[0]; pairs 2-7 stream
            # through the transient "sc" ring inside the first attention
            # piece, each arriving a few key-chunks ahead of its consumer.
            def v_proj_pair(pr, pool, tg):
                vt_t = sp.tile([128, 8, 256], bf16, tag="vt", bufs=2,
                               name=f"vt{pr}")
                nc.gpsimd.dma_start(
                    vt_t[:], band("vt", None)[:, :, 256 * pr:256 * (pr + 1)])
                v_ps = pool.tile([128, 2 * NQ], f32, tag=tg, bufs=2,
                                 name=f"vps{pr}")
                slices = [v_ps[:, 0:256], v_ps[:, 512:768]]
                for i in range(2):
                    for ci in range(8):
                        nc.tensor.matmul(
                            slices[i],
                            vt_t[:, ci, 128 * i:128 * (i + 1)],
                            wv_sb[:, ci, :],
                            start=(ci == 0), stop=False)
                    nc.tensor.matmul(slices[i], onesrow_sb[:], bvrow_sb[:],
                                     start=False, stop=True)
                    sc = 2 * pr + i
                    nc.vector.tensor_copy(
                        VA[sc].rearrange("p (h x) -> p h x", h=4)[:, :, 0:64],
                        slices[i].rearrange("p (h d) -> p h d", h=4))

            v_proj_pair(0, psC, "pv")
            v_proj_pair(1, psC, "pv")
            nc.gpsimd.dma_start(wo_sb[:], band("wo", None))

            # mask follows the projection weights on the scalar queue, in
            # column-half quarters: the first attention pieces only touch
            # query columns 0:SCW, so only half the mask is startup-critical
            for ch_ in range(NPC):
                for q in range(4):
                    nc.scalar.dma_start(
                        mask_sb[:, 4 * q:4 * q + 4, SCW * ch_:SCW * (ch_ + 1)],
                        band("mask", None)[:, 4 * q:4 * q + 4,
                                           SCW * ch_:SCW * (ch_ + 1)])

            # ---------------- attention ----------------
            def fa_load(hi, nb):
                t = sp.tile([128, 2, NQ], bf16, tag="fa", bufs=5,
                            name=f"fa{hi}_{nb}")
                fq = featsall[(hi, nb // (SCW // NQ))]
                src = bass.AP(
                    fq.tensor,
                    fq.offset + NQ * (nb % (SCW // NQ)),
                    [[SCW, 128], [128 * SCW, 2], [1, NQ]])
                # split across two queues so a gather-blocked load doesn't
                # stall the other quarters' loads
                eng = nc.sync if hi < 2 else nc.scalar
                eng.dma_start(t[:], src)
                return t

            fts = {}
            for hi in range(4):
                if hi == 3:
                    # quarters 0-2 are gathered by now: pre-load their first
                    # feature tiles so the output projection's early matmuls
                    # can run as soon as the rings free up
                    for (phi, pnb) in [(0, 0), (1, 0), (2, 0), (0, 1), (1, 1)]:
                        fts[(phi, pnb)] = fa_load(phi, pnb)
                for pc in range(NPC):
                    PV = psC.tile([128, SCW], f32, tag="pv", bufs=2,
                                  name=f"pv{hi}_{pc}")
                    for kc in range(KC):
                        SC = psB.tile([128, SCW], f32, tag="sc", bufs=2,
                                      name=f"sc{hi}_{pc}_{kc}")
                        for qb in range(SCW // NQ):
                            q0 = SCW * pc + NQ * qb
                            nc.tensor.matmul(
                                SC[:, NQ * qb:NQ * (qb + 1)],
                                KTs(hi)[:, 128 * kc:128 * (kc + 1)],
                                QTs(hi)[:, q0:q0 + NQ],
                                start=True, stop=True)
                        E = sp.tile([128, SCW], bf16, tag="e", bufs=2,
                                    name=f"e{hi}_{pc}_{kc}")
                        nc.scalar.activation(E[:], SC[:], Exp,
                                             scale=1.0 / np.sqrt(DH))
                        Dt = sp.tile([128, SCW], bf16, tag="d", bufs=3,
                                     name=f"d{hi}_{pc}_{kc}")
                        nc.vector.tensor_mul(
                            Dt[:], E[:],
                            mask_sb[:, kc, SCW * pc:SCW * (pc + 1)])
                        for qb in range(SCW // NQ):
                            nc.tensor.matmul(
                                PV[0:65, NQ * qb:NQ * (qb + 1)],
                                VA[kc][:, 65 * hi:65 * hi + 65],
                                Dt[:, NQ * qb:NQ * (qb + 1)],
                                start=(kc == 0), stop=(kc == KC - 1))
                        if hi == 0 and pc == 0 and kc % 2 == 0:
                            if kc < 12:
                                # V pairs 2-7 stream through the sc ring
                                v_proj_pair(2 + kc // 2, psB, "sc")
                            else:
                                # Q chunks p4-7 arrive before piece pc=1
                                q_proj_g4(2 + (kc - 12) // 2)

                    # normalize piece: R = 1/Z, broadcast to 64 partitions
                    # by a rank-1 PE matmul (ones ⊗ R) overwriting the PV
                    # psum rows just evacuated to PVs
                    PVs = sp.tile([65, SCW], f32, tag="pvs", bufs=2,
                                  name=f"pvs{hi}_{pc}")
                    nc.vector.tensor_copy(PVs[:], PV[0:65, :])
                    R = sp.tile([1, SCW], f32r, tag="r", bufs=1,
                                name=f"r{hi}_{pc}")
                    with nc.allow_low_precision(
                            reason="1/Z rank-1 broadcast rides fp32r matmul"):
                        nc.vector.reciprocal(R[:], PVs[64:65, :])
                    for qb in range(SCW // NQ):
                        nc.tensor.matmul(
                            PV[0:64, NQ * qb:NQ * (qb + 1)],
                            onesrow_sb[:, 0:64],
                            R[:, NQ * qb:NQ * (qb + 1)],
                            start=True, stop=True)
                    ft = sp.tile([64, SCW], bf16, tag="ft", bufs=2,
                                 name=f"ft{hi}_{pc}")
                    nc.vector.tensor_mul(ft[:], PVs[0:64, :], PV[0:64, :])
                    nc.sync.dma_start(feats_dram[(hi, pc)][:], ft[:])

                    # gather this piece right away: every gather except the
                    # final head's last piece completes under attention
                    if stub_collective:
                        for r in range(4):
                            nc.sync.dma_start(
                                featsall[(hi, pc)][64 * r:64 * (r + 1), :],
                                feats_dram[(hi, pc)][:])
                    else:
                        nc.gpsimd.collective_compute(
                            "AllGather",
                            mybir.AluOpType.bypass,
                            replica_groups=[[0, 1, 2, 3], [4, 5, 6, 7]],
                            ins=[feats_dram[(hi, pc)].opt()],
                            outs=[featsall[(hi, pc)].opt()],
                        )

            # ---------------- output projection ----------------
            # featsall[hi] rows 64*g+d hold global channel 256*g + 64*hi + d;
            # the host packs woT rows in matching (hi, g, d) order, so
            # contraction block ci = 2*hi + G covers featsall[hi] row pair G.
            for nb in range(S // NQ):
                for hi in range(4):
                    if (hi, nb) not in fts:
                        fts[(hi, nb)] = fa_load(hi, nb)
                for p in range(2):
                    pool, tg = (psB, "sc") if p == 0 else (psC, "pv")
                    pso = pool.tile([128, SCW], f32, tag=tg, bufs=2,
                                    name=f"pso{nb}_{p}")[:, 0:NQ]
                    n_ = 0
                    for hi in range(4):
                        for G in range(2):
                            ci = 2 * hi + G
                            nc.tensor.matmul(
                                pso,
                                wo_sb[:, ci, 128 * p:128 * (p + 1)],
                                fts[(hi, nb)][:, G, :],
                                start=(n_ == 0), stop=(n_ == 7))
                            n_ += 1
                    osb = sp.tile([128, NQ], f32, tag="osb", bufs=2,
                                  name=f"osb{nb}_{p}")
                    nc.vector.tensor_scalar_add(osb[:], pso,
                                                bias_sb[:, 10 + p:11 + p])
                    nc.sync.dma_start(
                        out_d[128 * p:128 * (p + 1), NQ * nb:NQ * (nb + 1)],
                        osb[:])

    nc.compile()
    return nc


def _get_module(S):
    if S not in _MODULES:
        _MODULES[S] = build_module(S)
    return _MODULES[S]


def _block(a, n):
    """[n*128, W] -> [128, n, W] (partition-major blocking)."""
    w = a.shape[1]
    return np.ascontiguousarray(
        a.reshape(n, 128, w).transpose(1, 0, 2))


def host_shard(inputs, S):
    """Build the 8 per-core input maps from the full-size problem inputs."""
    q = np.asarray(inputs["queries"], dtype=np.float32)
    k = np.asarray(inputs["keys"], dtype=np.float32)
    v = np.asarray(inputs["values"], dtype=np.float32)
    mask = np.asarray(inputs["mask"])
    Wq = np.asarray(inputs["Wq"], dtype=np.float32)
    Wk = np.asarray(inputs["Wk"], dtype=np.float32)
    Wv = np.asarray(inputs["Wv"], dtype=np.float32)
    Wo = np.asarray(inputs["Wo"], dtype=np.float32)
    bq = np.asarray(inputs["bq"], dtype=np.float32)
    bk = np.asarray(inputs["bk"], dtype=np.float32)
    bv = np.asarray(inputs["bv"], dtype=np.float32)
    bo = np.asarray(inputs["bo"], dtype=np.float32)

    JJ = S // 16
    OFF, TOTW = _offsets(S)
    bf = ml_dtypes.bfloat16

    maskb = _block((1 - mask[0, 0]).T.astype(bf), 16)
    wq_b = _block(Wq.T.astype(bf), 8)
    kt_b = [_block(k[b].T.astype(bf), 8) for b in range(B)]
    vt_b = [_block(v[b].T.astype(bf), 8) for b in range(B)]

    in_maps = []
    for c in range(NCORES):
        b, g = divmod(c, 4)
        heads = 4 * g + np.arange(4)
        s_idx = (16 * np.arange(JJ)[None, :] + heads[:, None]).reshape(-1)
        qts_b = _block(q[b][s_idx].T.astype(bf), 8)   # (128, 8, 4*JJ)
        ch = slice(256 * g, 256 * g + 256)

        xb = np.empty((128, TOTW), dtype=bf)

        def put(name, arr):
            w = arr.shape[1] * arr.shape[2]
            xb[:, OFF[name]:OFF[name] + w] = arr.reshape(128, w)

        put("qts", qts_b)
        put("kt", kt_b[b])
        put("vt", vt_b[b])
        put("mask", maskb)
        put("wq", wq_b)
        put("wk", _block(Wk.T[:, ch].astype(bf), 8))
        put("wv", _block(Wv.T[:, ch].astype(bf), 8))
        # woT rows permuted to (hi, g, d) order to match the quarter-gathered
        # feature layout (see output projection comment in build_module)
        wo_idx = (64 * np.arange(4)[:, None, None]      # hi
                  + 256 * np.arange(4)[None, :, None]   # g
                  + np.arange(64)[None, None, :]).reshape(-1)
        put("wo", _block(Wo.T[wo_idx][:, ch].astype(bf), 8))

        xf = np.zeros((128, 400), dtype=np.float32)
        xf[:, 0:8] = bq.reshape(8, 128).T
        xf[:, 8:10] = bk[ch].reshape(2, 128).T
        xf[:, 10:12] = bo[ch].reshape(2, 128).T
        xf[0, 16:272] = bv[ch]
        xf[0, 272:400] = 1.0
        in_maps.append({"xb": xb, "xf": xf})
    return in_maps


def assemble(results, S):
    out = np.empty((B, S, D), dtype=np.float32)
    for c in range(NCORES):
        b, g = divmod(c, 4)
        out[b, :, 256 * g:256 * g + 256] = results[c]["ofinal"].T
    return out


def kernel(**inputs):
    S = int(np.asarray(inputs["queries"]).shape[1])
    nc = _get_module(S)
    in_maps = host_shard(inputs, S)
    res = run_bass_kernel_spmd(nc, in_maps, core_ids=list(range(NCORES)))
    return assemble(res.results, S)


# revision 36
# speedup vs baseline: 1.0095x; 1.0095x over previous
"""Multi-head attention (with the reference's double-split_proj quirk) on 8
Trainium2 NeuronCores via Bass/Tile.

Sharding: core c handles batch b = c//4 and heads {4g..4g+3} where g = c%4
(data parallel on B, tensor parallel on heads). The double application of
_split_proj in the reference means the Q used for head i at attention row
j = (S/16)*h' + jj is q_proj[b, 16*jj + i, 64*h' + d]; per head that is a
gather of rows s ≡ i (mod 16) of q_proj over all 16 channel groups, so each
core only needs the 1/4 of query rows with s mod 16 in its head range —
sliced on the host. Everything runs transposed (channels on partitions,
positions on free): scores^T = K^T-chunks @ Q^T, softmax statistics come
from an appended ones-column on V (row 64 of the P@V output is the rowsum
Z), masking is a post-exp multiply by (1-mask)^T in bf16, and the output
projection consumes an AllGather of per-core attention features over each
batch's 4 cores.

v2 layout: all per-core device inputs travel as TWO dram tensors — one bf16
blob [128, TOTW] holding qts|kT|vT|mask|wq|wk|wv|wo pre-blocked on the host
into [128 partitions, ci, w] bands, and one tiny f32 blob for the biases —
so the whole input side is ~14 large DMAs instead of ~250 small ones
(per-DMA issue overhead on the DGE queues was a first-order cost), and the
per-dispatch PJRT argument marshaling drops from 13 buffers to 2.

Schedule: K projection and the first half of Q projection run up front
(psB "sc" psum ring); the remaining Q chunks and V-projection pairs
stream through the same transient ring inside the first attention piece
so PE never blocks on the slower input DMA. Attention runs head-major in
[SCW]-wide query pieces; each piece accumulates P@V over the 16 key
chunks in its own pv-ring slot, then normalizes with a DVE reciprocal on
the Z row followed by a rank-1 PE matmul (ones ⊗ 1/Z) that broadcasts
the reciprocal across partitions into the just-evacuated PV psum rows
(no DRAM round-trip, no cross-partition ops). Features AllGather
per (head, piece) — eight 128 KB gathers, of which all but the final
head's last piece complete while attention is still running — and the
earlier quarters' feature tiles are pre-loaded so the output projection's
tail only waits on the last 1024 query columns of one head.

Precision: projected Q^T/K^T stay float32r (full-rate fp32) so the scores
matmul keeps fp32-class precision; P@V runs bf16 (mask multiply on VectorE
in bf16 2x mode); all accumulation fp32 in PSUM.
"""

import sys

for _p in ("/opt/trn_rl_repo",):
    if _p not in sys.path:
        sys.path.append(_p)

import numpy as np
import ml_dtypes

import concourse.bass as bass
import concourse.bacc as bacc
import concourse.mybir as mybir
import concourse.tile as tile
from concourse.bass_utils import run_bass_kernel_spmd

B = 2
D = 1024
H = 16
DH = 64
NCORES = 8
S_FULL = 2048

f32 = mybir.dt.float32
f32r = mybir.dt.float32r
bf16 = mybir.dt.bfloat16

_MODULES = {}


def _offsets(S):
    """Column offsets (elements) of each band in the bf16 blob."""
    SQ = S // 4
    off = {}
    cur = 0
    for name, w in [
        ("qts", 8 * SQ), ("kt", 8 * S), ("vt", 8 * S), ("mask", 16 * S),
        ("wq", 8 * D), ("wk", 8 * 256), ("wv", 8 * 256), ("wo", 8 * 256),
    ]:
        off[name] = cur
        cur += w
    return off, cur


def build_module(S, stub_collective=False):
    """Build + compile the per-core Bass module (same program on all cores).

    stub_collective=True replaces the AllGather with local DMAs (wrong
    results on cores > 0) so the module is single-core analyzable.
    """
    JJ = S // 16          # rows per (head, channel-group)
    SQ = 4 * JJ           # host-gathered query rows per core (4 heads)
    KC = S // 128         # number of 128-wide key chunks
    SCW = 1024 if S >= 2048 else S   # query-piece width
    NPC = S // SCW        # pieces per head
    NQ = 512              # f32r moving-operand chunk
    OFF, TOTW = _offsets(S)

    nc = bacc.Bacc("TRN2", target_bir_lowering=False, debug=False,
                   num_devices=NCORES)

    xb_d = nc.dram_tensor("xb", [128, TOTW], bf16, kind="ExternalInput")
    xf_d = nc.dram_tensor("xf", [128, 400], f32, kind="ExternalInput")
    out_d = nc.dram_tensor("ofinal", [256, S], f32, kind="ExternalOutput")

    def band(name, ci):
        w = {"qts": SQ, "kt": S, "vt": S, "mask": S,
             "wq": D, "wk": 256, "wv": 256, "wo": 256}[name]
        n = 16 if name == "mask" else 8
        v = xb_d[:, OFF[name]:OFF[name] + n * w].rearrange(
            "p (c w) -> p c w", c=n)
        return v if ci is None else v[:, ci, :]

    Exp = mybir.ActivationFunctionType.Exp
    Ident = mybir.ActivationFunctionType.Identity

    with tile.TileContext(nc) as tc:
        with (
            tc.tile_pool(name="persist", bufs=1) as pp,
            tc.tile_pool(name="stream", bufs=1) as sp,
            tc.tile_pool(name="dram", bufs=1, space="DRAM") as dp,
            tc.tile_pool(name="psB", bufs=1, space="PSUM") as psB,
            tc.tile_pool(name="psC", bufs=1, space="PSUM") as psC,
        ):
            # ------------- resident inputs, ordered by criticality -------------
            # scalar queue: wk -> qts -> wq halves (Q/K projection inputs)
            # sync queue:   kt0 -> bias -> kt1-3 -> mask quarters
            # gpsimd queue: wv -> vt pairs (in v_proj_pair) -> wo
            wk_sb = pp.tile([128, 8, 256], bf16, tag="wk")
            nc.scalar.dma_start(wk_sb[:], band("wk", None))
            qts_sb = pp.tile([128, 8, SQ], bf16, tag="qts")
            nc.scalar.dma_start(qts_sb[:], band("qts", None))
            wq_sb = pp.tile([128, 8, D], bf16, tag="wq")
            for wh in range(2):
                nc.scalar.dma_start(
                    wq_sb[:, :, 512 * wh:512 * (wh + 1)],
                    band("wq", None)[:, :, 512 * wh:512 * (wh + 1)])
            wv_sb = pp.tile([128, 8, 256], bf16, tag="wv")
            nc.gpsimd.dma_start(wv_sb[:], band("wv", None))

            bias_sb = pp.tile([128, 16], f32, tag="bias")
            bvrow_sb = pp.tile([1, 256], f32r, tag="bvrow")
            onesrow_sb = pp.tile([1, 128], f32r, tag="onesrow")

            mask_sb = pp.tile([128, KC, S], bf16, tag="mask")
            wo_sb = pp.tile([128, 8, 256], bf16, tag="wo")

            # per-head Q^T and K^T packed in pairs: head hi lives on
            # partitions 64*(hi%2) .. +64 of pair tile hi//2
            QTp = [pp.tile([128, S], f32r, tag=f"QTp{h}", name=f"QTp{h}")
                   for h in range(2)]
            KTp = [pp.tile([128, S], f32r, tag=f"KTp{h}", name=f"KTp{h}")
                   for h in range(2)]

            def QTs(hi):
                return QTp[hi // 2][64 * (hi % 2):64 * (hi % 2) + 64, :]

            def KTs(hi):
                return KTp[hi // 2][64 * (hi % 2):64 * (hi % 2) + 64, :]

            # V+bias per key chunk, augmented with a ones column per head:
            # cols 65*hi+d (d<64), ones at 65*hi+64
            VA = [pp.tile([128, 260], bf16, tag=f"VA{sc}", name=f"VA{sc}")
                  for sc in range(KC)]
            for sc in range(KC):
                nc.vector.memset(
                    VA[sc].rearrange("p (h x) -> p h x", h=4)[:, :, 64:65], 1.0)

            # per-(head, piece) gather buffers: the gather granule is one
            # piece, so only the very last piece's gather sits in the tail
            feats_dram = {(x, pc): dp.tile([64, SCW], bf16,
                                           tag=f"feats_dram{x}_{pc}",
                                           name=f"feats_dram{x}_{pc}")
                          for x in range(4) for pc in range(NPC)}
            featsall = {(x, pc): dp.tile([256, SCW], bf16,
                                         tag=f"featsall{x}_{pc}",
                                         name=f"featsall{x}_{pc}")
                        for x in range(4) for pc in range(NPC)}

            # ---------------- K projection ----------------
            # kt ring: [128, 8, 512] chunks of kT (8 contraction blocks x
            # 512 key positions)
            for nb in range(S // NQ):
                kt_t = sp.tile([128, 8, NQ], bf16, tag="kt", bufs=2,
                               name=f"kt{nb}")
                nc.sync.dma_start(
                    kt_t[:], band("kt", None)[:, :, NQ * nb:NQ * (nb + 1)])
                if nb == 0:
                    nc.sync.dma_start(bias_sb[:], xf_d[:, 0:16])
                    nc.sync.dma_start(bvrow_sb[:],
                                      xf_d[0:1, 16:272].bitcast(f32r))
                    nc.sync.dma_start(onesrow_sb[:],
                                      xf_d[0:1, 272:400].bitcast(f32r))
                k_ps = psB.tile([128, 2 * NQ], f32, tag="sc", bufs=2,
                                name=f"kps{nb}")
                for p in range(2):
                    for ci in range(8):
                        nc.tensor.matmul(
                            k_ps[:, NQ * p:NQ * (p + 1)],
                            wk_sb[:, ci, 128 * p:128 * (p + 1)],
                            kt_t[:, ci, :],
                            start=(ci == 0), stop=(ci == 7))
                for p in range(2):
                    for half in range(2):
                        hi = 2 * p + half
                        nc.vector.tensor_scalar_add(
                            KTs(hi)[:, NQ * nb:NQ * (nb + 1)],
                            k_ps[64 * half:64 * half + 64, NQ * p:NQ * (p + 1)],
                            bias_sb[64 * half:64 * half + 64, 8 + p:9 + p])

            # ---------------- Q projection (half now, half mid-attention) ---
            def q_proj_g4(g4):
                q_ps = psB.tile([128, 2 * NQ], f32, tag="sc", bufs=2,
                                name=f"qps{g4}")
                for j in range(2):
                    po = 2 * g4 + j
                    for ci in range(8):
                        nc.tensor.matmul(
                            q_ps[:, NQ * j:NQ * (j + 1)],
                            wq_sb[:, ci, 128 * po:128 * (po + 1)],
                            qts_sb[:, ci, :],
                            start=(ci == 0), stop=(ci == 7))
                for j in range(2):
                    po = 2 * g4 + j
                    for hi in range(4):
                        for half in range(2):
                            h2 = 2 * po + half
                            nc.vector.tensor_scalar_add(
                                QTs(hi)[:, JJ * h2:JJ * (h2 + 1)],
                                q_ps[64 * half:64 * half + 64,
                                     NQ * j + JJ * hi:NQ * j + JJ * (hi + 1)],
                                bias_sb[64 * half:64 * half + 64, po:po + 1])

            # chunks p0-3 feed the first attention pieces (query cols 0:1024)
            q_proj_g4(0)
            q_proj_g4(1)

            # ---------------- V projection ----------------
            # psum [128, 1024] with two 256-wide slices at 512 spacing so the
            # concurrent accumulation groups sit in different PSUM banks.
            # Pairs 0-1 land before attention needs VA[0]; pairs 2-7 stream
            # through the transient "sc" ring inside the first attention
            # piece, each arriving a few key-chunks ahead of its consumer.
            def v_proj_pair(pr, pool, tg):
                vt_t = sp.tile([128, 8, 256], bf16, tag="vt", bufs=2,
                               name=f"vt{pr}")
                nc.gpsimd.dma_start(
                    vt_t[:], band("vt", None)[:, :, 256 * pr:256 * (pr + 1)])
                v_ps = pool.tile([128, 2 * NQ], f32, tag=tg, bufs=2,
                                 name=f"vps{pr}")
                slices = [v_ps[:, 0:256], v_ps[:, 512:768]]
                for i in range(2):
                    for ci in range(8):
                        nc.tensor.matmul(
                            slices[i],
                            vt_t[:, ci, 128 * i:128 * (i + 1)],
                            wv_sb[:, ci, :],
                            start=(ci == 0), stop=False)
                    nc.tensor.matmul(slices[i], onesrow_sb[:], bvrow_sb[:],
                                     start=False, stop=True)
                    sc = 2 * pr + i
                    nc.vector.tensor_copy(
                        VA[sc].rearrange("p (h x) -> p h x", h=4)[:, :, 0:64],
                        slices[i].rearrange("p (h d) -> p h d", h=4))

            v_proj_pair(0, psC, "pv")
            v_proj_pair(1, psC, "pv")
            nc.gpsimd.dma_start(wo_sb[:], band("wo", None))

            # mask follows the projection weights on the scalar queue, in
            # column-half quarters: the first attention pieces only touch
            # query columns 0:SCW, so only half the mask is startup-critical
            for ch_ in range(NPC):
                for q in range(4):
                    nc.scalar.dma_start(
                        mask_sb[:, 4 * q:4 * q + 4, SCW * ch_:SCW * (ch_ + 1)],
                        band("mask", None)[:, 4 * q:4 * q + 4,
                                           SCW * ch_:SCW * (ch_ + 1)])

            # ---------------- attention ----------------
            def fa_load(hi, nb):
                t = sp.tile([128, 2, NQ], bf16, tag="fa", bufs=5,
                            name=f"fa{hi}_{nb}")
                fq = featsall[(hi, nb // (SCW // NQ))]
                src = bass.AP(
                    fq.tensor,
                    fq.offset + NQ * (nb % (SCW // NQ)),
                    [[SCW, 128], [128 * SCW, 2], [1, NQ]])
                # split across two queues so a gather-blocked load doesn't
                # stall the other quarters' loads
                eng = nc.sync if hi < 2 else nc.scalar
                eng.dma_start(t[:], src)
                return t

            fts = {}
            for hi in range(4):
                if hi == 3:
                    # quarters 0-2 are gathered by now: pre-load their first
                    # feature tiles so the output projection's early matmuls
                    # can run as soon as the rings free up
                    for (phi, pnb) in [(0, 0), (1, 0), (2, 0), (0, 1), (1, 1)]:
                        fts[(phi, pnb)] = fa_load(phi, pnb)
                for pc in range(NPC):
                    PV = psC.tile([128, SCW], f32, tag="pv", bufs=2,
                                  name=f"pv{hi}_{pc}")
                    for kc in range(KC):
                        SC = psB.tile([128, SCW], f32, tag="sc", bufs=2,
                                      name=f"sc{hi}_{pc}_{kc}")
                        for qb in range(SCW // NQ):
                            q0 = SCW * pc + NQ * qb
                            nc.tensor.matmul(
                                SC[:, NQ * qb:NQ * (qb + 1)],
                                KTs(hi)[:, 128 * kc:128 * (kc + 1)],
                                QTs(hi)[:, q0:q0 + NQ],
                                start=True, stop=True)
                        E = sp.tile([128, SCW], bf16, tag="e", bufs=2,
                                    name=f"e{hi}_{pc}_{kc}")
                        nc.scalar.activation(E[:], SC[:], Exp,
                                             scale=1.0 / np.sqrt(DH))
                        Dt = sp.tile([128, SCW], bf16, tag="d", bufs=3,
                                     name=f"d{hi}_{pc}_{kc}")
                        nc.vector.tensor_mul(
                            Dt[:], E[:],
                            mask_sb[:, kc, SCW * pc:SCW * (pc + 1)])
                        for qb in range(SCW // NQ):
                            nc.tensor.matmul(
                                PV[0:65, NQ * qb:NQ * (qb + 1)],
                                VA[kc][:, 65 * hi:65 * hi + 65],
                                Dt[:, NQ * qb:NQ * (qb + 1)],
                                start=(kc == 0), stop=(kc == KC - 1))
                        if hi == 0 and pc == 0 and kc % 2 == 0:
                            if kc < 12:
                                # V pairs 2-7 stream through the sc ring
                                v_proj_pair(2 + kc // 2, psB, "sc")
                            else:
                                # Q chunks p4-7 arrive before piece pc=1
                                q_proj_g4(2 + (kc - 12) // 2)

                    # normalize piece: R = 1/Z, broadcast to 64 partitions
                    # by a rank-1 PE matmul (ones ⊗ R) overwriting the PV
                    # psum rows just evacuated to PVs
                    PVs = sp.tile([65, SCW], f32, tag="pvs", bufs=2,
                                  name=f"pvs{hi}_{pc}")
                    nc.vector.tensor_copy(PVs[:], PV[0:65, :])
                    R = sp.tile([1, SCW], f32r, tag="r", bufs=1,
                                name=f"r{hi}_{pc}")
                    with nc.allow_low_precision(
                            reason="1/Z rank-1 broadcast rides fp32r matmul"):
                        nc.vector.reciprocal(R[:], PVs[64:65, :])
                    for qb in range(SCW // NQ):
                        nc.tensor.matmul(
                            PV[0:64, NQ * qb:NQ * (qb + 1)],
                            onesrow_sb[:, 0:64],
                            R[:, NQ * qb:NQ * (qb + 1)],
                            start=True, stop=True)
                    ft = sp.tile([64, SCW], bf16, tag="ft", bufs=2,
                                 name=f"ft{hi}_{pc}")
                    nc.vector.tensor_mul(ft[:], PVs[0:64, :], PV[0:64, :])
                    nc.sync.dma_start(feats_dram[(hi, pc)][:], ft[:])

                    # gather this piece right away: every gather except the
                    # final head's last piece completes under attention
                    if stub_collective:
                        for r in range(4):
                            nc.sync.dma_start(
                                featsall[(hi, pc)][64 * r:64 * (r + 1), :],
                                feats_dram[(hi, pc)][:])
                    else:
                        nc.gpsimd.collective_compute(
                            "AllGather",
                            mybir.AluOpType.bypass,
                            replica_groups=[[0, 1, 2, 3], [4, 5, 6, 7]],
                            ins=[feats_dram[(hi, pc)].opt()],
                            outs=[featsall[(hi, pc)].opt()],
                        )

            # ---------------- output projection ----------------
            # featsall[hi] rows 64*g+d hold global channel 256*g + 64*hi + d;
            # the host packs woT rows in matching (hi, g, d) order, so
            # contraction block ci = 2*hi + G covers featsall[hi] row pair G.
            for nb in range(S // NQ):
                for hi in range(4):
                    if (hi, nb) not in fts:
                        fts[(hi, nb)] = fa_load(hi, nb)
                for p in range(2):
                    pool, tg = (psB, "sc") if p == 0 else (psC, "pv")
                    pso = pool.tile([128, SCW], f32, tag=tg, bufs=2,
                                    name=f"pso{nb}_{p}")[:, 0:NQ]
                    n_ = 0
                    for hi in range(4):
                        for G in range(2):
                            ci = 2 * hi + G
                            nc.tensor.matmul(
                                pso,
                                wo_sb[:, ci, 128 * p:128 * (p + 1)],
                                fts[(hi, nb)][:, G, :],
                                start=(n_ == 0), stop=(n_ == 7))
                            n_ += 1
                    osb = sp.tile([128, NQ], f32, tag="osb", bufs=2,
                                  name=f"osb{nb}_{p}")
                    nc.vector.tensor_scalar_add(osb[:], pso,
                                                bias_sb[:, 10 + p:11 + p])
                    nc.sync.dma_start(
                        out_d[128 * p:128 * (p + 1), NQ * nb:NQ * (nb + 1)],
                        osb[:])

    nc.compile()
    return nc


def _get_module(S):
    if S not in _MODULES:
        _MODULES[S] = build_module(S)
    return _MODULES[S]


def _block(a, n):
    """[n*128, W] -> [128, n, W] (partition-major blocking)."""
    w = a.shape[1]
    return np.ascontiguousarray(
        a.reshape(n, 128, w).transpose(1, 0, 2))


def host_shard(inputs, S):
    """Build the 8 per-core input maps from the full-size problem inputs."""
    q = np.asarray(inputs["queries"], dtype=np.float32)
    k = np.asarray(inputs["keys"], dtype=np.float32)
    v = np.asarray(inputs["values"], dtype=np.float32)
    mask = np.asarray(inputs["mask"])
    Wq = np.asarray(inputs["Wq"], dtype=np.float32)
    Wk = np.asarray(inputs["Wk"], dtype=np.float32)
    Wv = np.asarray(inputs["Wv"], dtype=np.float32)
    Wo = np.asarray(inputs["Wo"], dtype=np.float32)
    bq = np.asarray(inputs["bq"], dtype=np.float32)
    bk = np.asarray(inputs["bk"], dtype=np.float32)
    bv = np.asarray(inputs["bv"], dtype=np.float32)
    bo = np.asarray(inputs["bo"], dtype=np.float32)

    JJ = S // 16
    OFF, TOTW = _offsets(S)
    bf = ml_dtypes.bfloat16

    maskb = _block((1 - mask[0, 0]).T.astype(bf), 16)
    wq_b = _block(Wq.T.astype(bf), 8)
    kt_b = [_block(k[b].T.astype(bf), 8) for b in range(B)]
    vt_b = [_block(v[b].T.astype(bf), 8) for b in range(B)]

    in_maps = []
    for c in range(NCORES):
        b, g = divmod(c, 4)
        heads = 4 * g + np.arange(4)
        s_idx = (16 * np.arange(JJ)[None, :] + heads[:, None]).reshape(-1)
        qts_b = _block(q[b][s_idx].T.astype(bf), 8)   # (128, 8, 4*JJ)
        ch = slice(256 * g, 256 * g + 256)

        xb = np.empty((128, TOTW), dtype=bf)

        def put(name, arr):
            w = arr.shape[1] * arr.shape[2]
            xb[:, OFF[name]:OFF[name] + w] = arr.reshape(128, w)

        put("qts", qts_b)
        put("kt", kt_b[b])
        put("vt", vt_b[b])
        put("mask", maskb)
        put("wq", wq_b)
        put("wk", _block(Wk.T[:, ch].astype(bf), 8))
        put("wv", _block(Wv.T[:, ch].astype(bf), 8))
        # woT rows permuted to (hi, g, d) order to match the quarter-gathered
        # feature layout (see output projection comment in build_module)
        wo_idx = (64 * np.arange(4)[:, None, None]      # hi
                  + 256 * np.arange(4)[None, :, None]   # g
                  + np.arange(64)[None, None, :]).reshape(-1)
        put("wo", _block(Wo.T[wo_idx][:, ch].astype(bf), 8))

        xf = np.zeros((128, 400), dtype=np.float32)
        xf[:, 0:8] = bq.reshape(8, 128).T
        xf[:, 8:10] = bk[ch].reshape(2, 128).T
        xf[:, 10:12] = bo[ch].reshape(2, 128).T
        xf[0, 16:272] = bv[ch]
        xf[0, 272:400] = 1.0
        in_maps.append({"xb": xb, "xf": xf})
    return in_maps


def assemble(results, S):
    out = np.empty((B, S, D), dtype=np.float32)
    for c in range(NCORES):
        b, g = divmod(c, 4)
        out[b, :, 256 * g:256 * g + 256] = results[c]["ofinal"].T
    return out


def kernel(**inputs):
    S = int(np.asarray(inputs["queries"]).shape[1])
    nc = _get_module(S)
    in_maps = host_shard(inputs, S)
    res = run_bass_kernel_spmd(nc, in_maps, core_ids=list(range(NCORES)))
    return assemble(res.results, S)


# revision 39
# speedup vs baseline: 1.1017x; 1.0913x over previous
"""Multi-head attention (with the reference's double-split_proj quirk) on 8
Trainium2 NeuronCores via Bass/Tile.

Sharding: core c handles batch b = c//4 and heads {4g..4g+3} where g = c%4
(data parallel on B, tensor parallel on heads). The double application of
_split_proj in the reference means the Q used for head i at attention row
j = (S/16)*h' + jj is q_proj[b, 16*jj + i, 64*h' + d]; per head that is a
gather of rows s ≡ i (mod 16) of q_proj over all 16 channel groups, so each
core only needs the 1/4 of query rows with s mod 16 in its head range —
sliced on the host. Everything runs transposed (channels on partitions,
positions on free): scores^T = K^T-chunks @ Q^T, softmax statistics come
from an appended ones-column on V (row 64 of the P@V output is the rowsum
Z), masking is a post-exp multiply by (1-mask)^T in bf16, and the output
projection consumes an AllGather of per-core attention features over each
batch's 4 cores.

v2 layout: all per-core device inputs travel as TWO dram tensors — one bf16
blob [128, TOTW] holding qts|kT|vT|mask|wq|wk|wv|wo pre-blocked on the host
into [128 partitions, ci, w] bands, and one tiny f32 blob for the biases —
so the whole input side is ~14 large DMAs instead of ~250 small ones
(per-DMA issue overhead on the DGE queues was a first-order cost), and the
per-dispatch PJRT argument marshaling drops from 13 buffers to 2.

Schedule: K projection and the first half of Q projection run up front
(psB "sc" psum ring); the remaining Q chunks and V-projection pairs
stream through the same transient ring inside the first attention piece
so PE never blocks on the slower input DMA. Attention runs head-major in
[SCW]-wide query pieces; each piece accumulates P@V over the 16 key
chunks in its own pv-ring slot, then normalizes with a DVE reciprocal on
the Z row followed by a rank-1 PE matmul (ones ⊗ 1/Z) that broadcasts
the reciprocal across partitions into the just-evacuated PV psum rows
(no DRAM round-trip, no cross-partition ops). Features AllGather
per (head, piece) — eight 128 KB gathers, of which all but the final
head's last piece complete while attention is still running — and the
earlier quarters' feature tiles are pre-loaded so the output projection's
tail only waits on the last 1024 query columns of one head.

Precision: projected Q^T/K^T stay float32r (full-rate fp32) so the scores
matmul keeps fp32-class precision; P@V runs bf16 (mask multiply on VectorE
in bf16 2x mode); all accumulation fp32 in PSUM.
"""

import sys

for _p in ("/opt/trn_rl_repo",):
    if _p not in sys.path:
        sys.path.append(_p)

import numpy as np
import ml_dtypes

import concourse.bass as bass
import concourse.bacc as bacc
import concourse.mybir as mybir
import concourse.tile as tile
from concourse.bass_utils import run_bass_kernel_spmd

B = 2
D = 1024
H = 16
DH = 64
NCORES = 8
S_FULL = 2048

f32 = mybir.dt.float32
f32r = mybir.dt.float32r
bf16 = mybir.dt.bfloat16

_MODULES = {}


def _offsets(S):
    """Column offsets (elements) of each band in the bf16 blob."""
    SQ = S // 4
    off = {}
    cur = 0
    for name, w in [
        ("qts", 8 * SQ), ("kt", 8 * S), ("vt", 8 * S), ("mask", 16 * S),
        ("wq", 8 * D), ("wk", 8 * 256), ("wv", 8 * 256), ("wo", 8 * 256),
    ]:
        off[name] = cur
        cur += w
    return off, cur


def build_module(S, stub_collective=False):
    """Build + compile the per-core Bass module (same program on all cores).

    stub_collective=True replaces the AllGather with local DMAs (wrong
    results on cores > 0) so the module is single-core analyzable.
    """
    JJ = S // 16          # rows per (head, channel-group)
    SQ = 4 * JJ           # host-gathered query rows per core (4 heads)
    KC = S // 128         # number of 128-wide key chunks
    SCW = 1024 if S >= 2048 else S   # query-piece width
    NPC = S // SCW        # pieces per head
    NQ = 512              # f32r moving-operand chunk
    OFF, TOTW = _offsets(S)

    nc = bacc.Bacc("TRN2", target_bir_lowering=False, debug=False,
                   num_devices=NCORES)

    xb_d = nc.dram_tensor("xb", [128, TOTW], bf16, kind="ExternalInput")
    xf_d = nc.dram_tensor("xf", [128, 400], f32, kind="ExternalInput")
    out_d = nc.dram_tensor("ofinal", [256, S], f32, kind="ExternalOutput")

    def band(name, ci):
        w = {"qts": SQ, "kt": S, "vt": S, "mask": S,
             "wq": D, "wk": 256, "wv": 256, "wo": 256}[name]
        n = 16 if name == "mask" else 8
        v = xb_d[:, OFF[name]:OFF[name] + n * w].rearrange(
            "p (c w) -> p c w", c=n)
        return v if ci is None else v[:, ci, :]

    Exp = mybir.ActivationFunctionType.Exp
    Ident = mybir.ActivationFunctionType.Identity

    with tile.TileContext(nc) as tc:
        with (
            tc.tile_pool(name="persist", bufs=1) as pp,
            tc.tile_pool(name="stream", bufs=1) as sp,
            tc.tile_pool(name="dram", bufs=1, space="DRAM") as dp,
            tc.tile_pool(name="psB", bufs=1, space="PSUM") as psB,
            tc.tile_pool(name="psC", bufs=1, space="PSUM") as psC,
        ):
            # ------------- resident inputs, ordered by criticality -------------
            # scalar queue: wk -> qts -> wq halves (Q/K projection inputs)
            # sync queue:   kt0 -> bias -> kt1-3 -> mask quarters
            # gpsimd queue: wv -> vt pairs (in v_proj_pair) -> wo
            wk_sb = pp.tile([128, 8, 256], bf16, tag="wk")
            nc.scalar.dma_start(wk_sb[:], band("wk", None))
            qts_sb = pp.tile([128, 8, SQ], bf16, tag="qts")
            nc.scalar.dma_start(qts_sb[:], band("qts", None))
            wq_sb = pp.tile([128, 8, D], bf16, tag="wq")
            for wh in range(2):
                nc.scalar.dma_start(
                    wq_sb[:, :, 512 * wh:512 * (wh + 1)],
                    band("wq", None)[:, :, 512 * wh:512 * (wh + 1)])
            wv_sb = pp.tile([128, 8, 256], bf16, tag="wv")
            nc.gpsimd.dma_start(wv_sb[:], band("wv", None))

            bias_sb = pp.tile([128, 16], f32, tag="bias")
            bvrow_sb = pp.tile([1, 256], f32r, tag="bvrow")
            onesrow_sb = pp.tile([1, 128], f32r, tag="onesrow")

            mask_sb = pp.tile([128, KC, S], bf16, tag="mask")
            wo_sb = pp.tile([128, 8, 256], bf16, tag="wo")

            # per-head Q^T and K^T packed in pairs: head hi lives on
            # partitions 64*(hi%2) .. +64 of pair tile hi//2
            QTp = [pp.tile([128, S], f32r, tag=f"QTp{h}", name=f"QTp{h}")
                   for h in range(2)]
            KTp = [pp.tile([128, S], f32r, tag=f"KTp{h}", name=f"KTp{h}")
                   for h in range(2)]

            def QTs(hi):
                return QTp[hi // 2][64 * (hi % 2):64 * (hi % 2) + 64, :]

            def KTs(hi):
                return KTp[hi // 2][64 * (hi % 2):64 * (hi % 2) + 64, :]

            # V+bias per key chunk, augmented with a ones column per head:
            # cols 65*hi+d (d<64), ones at 65*hi+64
            VA = [pp.tile([128, 260], bf16, tag=f"VA{sc}", name=f"VA{sc}")
                  for sc in range(KC)]
            for sc in range(KC):
                nc.vector.memset(
                    VA[sc].rearrange("p (h x) -> p h x", h=4)[:, :, 64:65], 1.0)

            # per-(head, piece) gather buffers: the gather granule is one
            # piece, so only the very last piece's gather sits in the tail
            feats_dram = {(x, pc): dp.tile([64, SCW], bf16,
                                           tag=f"feats_dram{x}_{pc}",
                                           name=f"feats_dram{x}_{pc}")
                          for x in range(4) for pc in range(NPC)}
            featsall = {(x, pc): dp.tile([256, SCW], bf16,
                                         tag=f"featsall{x}_{pc}",
                                         name=f"featsall{x}_{pc}")
                        for x in range(4) for pc in range(NPC)}

            # ---------------- K projection ----------------
            # kt ring: [128, 8, 512] chunks of kT (8 contraction blocks x
            # 512 key positions)
            for nb in range(S // NQ):
                kt_t = sp.tile([128, 8, NQ], bf16, tag="kt", bufs=2,
                               name=f"kt{nb}")
                nc.sync.dma_start(
                    kt_t[:], band("kt", None)[:, :, NQ * nb:NQ * (nb + 1)])
                if nb == 0:
                    nc.sync.dma_start(bias_sb[:], xf_d[:, 0:16])
                    nc.sync.dma_start(bvrow_sb[:],
                                      xf_d[0:1, 16:272].bitcast(f32r))
                    nc.sync.dma_start(onesrow_sb[:],
                                      xf_d[0:1, 272:400].bitcast(f32r))
                k_ps = psB.tile([128, 2 * NQ], f32, tag="sc", bufs=2,
                                name=f"kps{nb}")
                for p in range(2):
                    for ci in range(8):
                        nc.tensor.matmul(
                            k_ps[:, NQ * p:NQ * (p + 1)],
                            wk_sb[:, ci, 128 * p:128 * (p + 1)],
                            kt_t[:, ci, :],
                            start=(ci == 0), stop=(ci == 7))
                for p in range(2):
                    for half in range(2):
                        hi = 2 * p + half
                        nc.vector.tensor_scalar_add(
                            KTs(hi)[:, NQ * nb:NQ * (nb + 1)],
                            k_ps[64 * half:64 * half + 64, NQ * p:NQ * (p + 1)],
                            bias_sb[64 * half:64 * half + 64, 8 + p:9 + p])

            # ---------------- Q projection (half now, half mid-attention) ---
            def q_proj_g4(g4):
                q_ps = psB.tile([128, 2 * NQ], f32, tag="sc", bufs=2,
                                name=f"qps{g4}")
                for j in range(2):
                    po = 2 * g4 + j
                    for ci in range(8):
                        nc.tensor.matmul(
                            q_ps[:, NQ * j:NQ * (j + 1)],
                            wq_sb[:, ci, 128 * po:128 * (po + 1)],
                            qts_sb[:, ci, :],
                            start=(ci == 0), stop=(ci == 7))
                for j in range(2):
                    po = 2 * g4 + j
                    for hi in range(4):
                        for half in range(2):
                            h2 = 2 * po + half
                            nc.vector.tensor_scalar_add(
                                QTs(hi)[:, JJ * h2:JJ * (h2 + 1)],
                                q_ps[64 * half:64 * half + 64,
                                     NQ * j + JJ * hi:NQ * j + JJ * (hi + 1)],
                                bias_sb[64 * half:64 * half + 64, po:po + 1])

            # chunks p0-3 feed the first attention pieces (query cols 0:1024)
            q_proj_g4(0)
            q_proj_g4(1)

            # ---------------- V projection ----------------
            # psum [128, 1024] with two 256-wide slices at 512 spacing so the
            # concurrent accumulation groups sit in different PSUM banks.
            # Pairs 0-1 land before attention needs VA[0]; pairs 2-7 stream
            # through the transient "sc" ring inside the first attention
            # piece, each arriving a few key-chunks ahead of its consumer.
            def v_proj_pair(pr, pool, tg):
                vt_t = sp.tile([128, 8, 256], bf16, tag="vt", bufs=2,
                               name=f"vt{pr}")
                nc.gpsimd.dma_start(
                    vt_t[:], band("vt", None)[:, :, 256 * pr:256 * (pr + 1)])
                v_ps = pool.tile([128, 2 * NQ], f32, tag=tg, bufs=2,
                                 name=f"vps{pr}")
                slices = [v_ps[:, 0:256], v_ps[:, 512:768]]
                for i in range(2):
                    for ci in range(8):
                        nc.tensor.matmul(
                            slices[i],
                            vt_t[:, ci, 128 * i:128 * (i + 1)],
                            wv_sb[:, ci, :],
                            start=(ci == 0), stop=False)
                    nc.tensor.matmul(slices[i], onesrow_sb[:], bvrow_sb[:],
                                     start=False, stop=True)
                    sc = 2 * pr + i
                    nc.vector.tensor_copy(
                        VA[sc].rearrange("p (h x) -> p h x", h=4)[:, :, 0:64],
                        slices[i].rearrange("p (h d) -> p h d", h=4))

            v_proj_pair(0, psC, "pv")
            v_proj_pair(1, psC, "pv")
            nc.gpsimd.dma_start(wo_sb[:], band("wo", None))

            # mask follows the projection weights on the scalar queue, in
            # column-half quarters: the first attention pieces only touch
            # query columns 0:SCW, so only half the mask is startup-critical
            for ch_ in range(NPC):
                for q in range(4):
                    nc.scalar.dma_start(
                        mask_sb[:, 4 * q:4 * q + 4, SCW * ch_:SCW * (ch_ + 1)],
                        band("mask", None)[:, 4 * q:4 * q + 4,
                                           SCW * ch_:SCW * (ch_ + 1)])

            # ---------------- attention ----------------
            def fa_load(hi, nb):
                t = sp.tile([128, 2, NQ], bf16, tag="fa", bufs=4,
                            name=f"fa{hi}_{nb}")
                fq = featsall[(hi, nb // (SCW // NQ))]
                src = bass.AP(
                    fq.tensor,
                    fq.offset + NQ * (nb % (SCW // NQ)),
                    [[SCW, 128], [128 * SCW, 2], [1, NQ]])
                # split across two queues so a gather-blocked load doesn't
                # stall the other quarters' loads
                eng = nc.sync if hi < 2 else nc.scalar
                eng.dma_start(t[:], src)
                return t

            fts = {}
            for hi in range(4):
                if hi == 3:
                    # quarters 0-2 are gathered by now: pre-load their first
                    # feature tiles so the output projection's early matmuls
                    # can run as soon as the rings free up
                    for (phi, pnb) in [(0, 0), (1, 0), (2, 0), (0, 1)]:
                        fts[(phi, pnb)] = fa_load(phi, pnb)
                for pc in range(NPC):
                    PV = psC.tile([128, SCW], f32, tag="pv", bufs=2,
                                  name=f"pv{hi}_{pc}")
                    for kc in range(KC):
                        SC = psB.tile([128, SCW], f32, tag="sc", bufs=2,
                                      name=f"sc{hi}_{pc}_{kc}")
                        for qb in range(SCW // NQ):
                            q0 = SCW * pc + NQ * qb
                            nc.tensor.matmul(
                                SC[:, NQ * qb:NQ * (qb + 1)],
                                KTs(hi)[:, 128 * kc:128 * (kc + 1)],
                                QTs(hi)[:, q0:q0 + NQ],
                                start=True, stop=True)
                        E = sp.tile([128, SCW], bf16, tag="e", bufs=3,
                                    name=f"e{hi}_{pc}_{kc}")
                        nc.scalar.activation(E[:], SC[:], Exp,
                                             scale=1.0 / np.sqrt(DH))
                        Dt = sp.tile([128, SCW], bf16, tag="d", bufs=3,
                                     name=f"d{hi}_{pc}_{kc}")
                        nc.vector.tensor_mul(
                            Dt[:], E[:],
                            mask_sb[:, kc, SCW * pc:SCW * (pc + 1)])
                        for qb in range(SCW // NQ):
                            nc.tensor.matmul(
                                PV[0:65, NQ * qb:NQ * (qb + 1)],
                                VA[kc][:, 65 * hi:65 * hi + 65],
                                Dt[:, NQ * qb:NQ * (qb + 1)],
                                start=(kc == 0), stop=(kc == KC - 1))
                        if hi == 0 and pc == 0 and kc % 2 == 0:
                            if kc < 12:
                                # V pairs 2-7 stream through the sc ring
                                v_proj_pair(2 + kc // 2, psB, "sc")
                            else:
                                # Q chunks p4-7 arrive before piece pc=1
                                q_proj_g4(2 + (kc - 12) // 2)

                    # normalize piece: R = 1/Z, broadcast to 64 partitions
                    # by a rank-1 PE matmul (ones ⊗ R) overwriting the PV
                    # psum rows just evacuated to PVs
                    PVs = sp.tile([65, SCW], f32, tag="pvs", bufs=2,
                                  name=f"pvs{hi}_{pc}")
                    nc.vector.tensor_copy(PVs[:], PV[0:65, :])
                    R = sp.tile([1, SCW], f32r, tag="r", bufs=1,
                                name=f"r{hi}_{pc}")
                    with nc.allow_low_precision(
                            reason="1/Z rank-1 broadcast rides fp32r matmul"):
                        nc.vector.reciprocal(R[:], PVs[64:65, :])
                    for qb in range(SCW // NQ):
                        nc.tensor.matmul(
                            PV[0:64, NQ * qb:NQ * (qb + 1)],
                            onesrow_sb[:, 0:64],
                            R[:, NQ * qb:NQ * (qb + 1)],
                            start=True, stop=True)
                    ft = sp.tile([64, SCW], bf16, tag="ft", bufs=2,
                                 name=f"ft{hi}_{pc}")
                    nc.vector.tensor_mul(ft[:], PVs[0:64, :], PV[0:64, :])
                    nc.sync.dma_start(feats_dram[(hi, pc)][:], ft[:])

                    # gather this piece right away: every gather except the
                    # final head's last piece completes under attention
                    if stub_collective:
                        for r in range(4):
                            nc.sync.dma_start(
                                featsall[(hi, pc)][64 * r:64 * (r + 1), :],
                                feats_dram[(hi, pc)][:])
                    else:
                        nc.gpsimd.collective_compute(
                            "AllGather",
                            mybir.AluOpType.bypass,
                            replica_groups=[[0, 1, 2, 3], [4, 5, 6, 7]],
                            ins=[feats_dram[(hi, pc)].opt()],
                            outs=[featsall[(hi, pc)].opt()],
                        )

            # ---------------- output projection ----------------
            # featsall[hi] rows 64*g+d hold global channel 256*g + 64*hi + d;
            # the host packs woT rows in matching (hi, g, d) order, so
            # contraction block ci = 2*hi + G covers featsall[hi] row pair G.
            for nb in range(S // NQ):
                for hi in range(4):
                    if (hi, nb) not in fts:
                        fts[(hi, nb)] = fa_load(hi, nb)
                for p in range(2):
                    pool, tg = (psB, "sc") if p == 0 else (psC, "pv")
                    pso = pool.tile([128, SCW], f32, tag=tg, bufs=2,
                                    name=f"pso{nb}_{p}")[:, 0:NQ]
                    n_ = 0
                    for hi in range(4):
                        for G in range(2):
                            ci = 2 * hi + G
                            nc.tensor.matmul(
                                pso,
                                wo_sb[:, ci, 128 * p:128 * (p + 1)],
                                fts[(hi, nb)][:, G, :],
                                start=(n_ == 0), stop=(n_ == 7))
                            n_ += 1
                    osb = sp.tile([128, NQ], f32, tag="osb", bufs=2,
                                  name=f"osb{nb}_{p}")
                    nc.vector.tensor_scalar_add(osb[:], pso,
                                                bias_sb[:, 10 + p:11 + p])
                    nc.sync.dma_start(
                        out_d[128 * p:128 * (p + 1), NQ * nb:NQ * (nb + 1)],
                        osb[:])

    nc.compile()
    return nc


def _get_module(S):
    if S not in _MODULES:
        _MODULES[S] = build_module(S)
    return _MODULES[S]


def _block(a, n):
    """[n*128, W] -> [128, n, W] (partition-major blocking)."""
    w = a.shape[1]
    return np.ascontiguousarray(
        a.reshape(n, 128, w).transpose(1, 0, 2))


def host_shard(inputs, S):
    """Build the 8 per-core input maps from the full-size problem inputs."""
    q = np.asarray(inputs["queries"], dtype=np.float32)
    k = np.asarray(inputs["keys"], dtype=np.float32)
    v = np.asarray(inputs["values"], dtype=np.float32)
    mask = np.asarray(inputs["mask"])
    Wq = np.asarray(inputs["Wq"], dtype=np.float32)
    Wk = np.asarray(inputs["Wk"], dtype=np.float32)
    Wv = np.asarray(inputs["Wv"], dtype=np.float32)
    Wo = np.asarray(inputs["Wo"], dtype=np.float32)
    bq = np.asarray(inputs["bq"], dtype=np.float32)
    bk = np.asarray(inputs["bk"], dtype=np.float32)
    bv = np.asarray(inputs["bv"], dtype=np.float32)
    bo = np.asarray(inputs["bo"], dtype=np.float32)

    JJ = S // 16
    OFF, TOTW = _offsets(S)
    bf = ml_dtypes.bfloat16

    maskb = _block((1 - mask[0, 0]).T.astype(bf), 16)
    wq_b = _block(Wq.T.astype(bf), 8)
    kt_b = [_block(k[b].T.astype(bf), 8) for b in range(B)]
    vt_b = [_block(v[b].T.astype(bf), 8) for b in range(B)]

    in_maps = []
    for c in range(NCORES):
        b, g = divmod(c, 4)
        heads = 4 * g + np.arange(4)
        s_idx = (16 * np.arange(JJ)[None, :] + heads[:, None]).reshape(-1)
        qts_b = _block(q[b][s_idx].T.astype(bf), 8)   # (128, 8, 4*JJ)
        ch = slice(256 * g, 256 * g + 256)

        xb = np.empty((128, TOTW), dtype=bf)

        def put(name, arr):
            w = arr.shape[1] * arr.shape[2]
            xb[:, OFF[name]:OFF[name] + w] = arr.reshape(128, w)

        put("qts", qts_b)
        put("kt", kt_b[b])
        put("vt", vt_b[b])
        put("mask", maskb)
        put("wq", wq_b)
        put("wk", _block(Wk.T[:, ch].astype(bf), 8))
        put("wv", _block(Wv.T[:, ch].astype(bf), 8))
        # woT rows permuted to (hi, g, d) order to match the quarter-gathered
        # feature layout (see output projection comment in build_module)
        wo_idx = (64 * np.arange(4)[:, None, None]      # hi
                  + 256 * np.arange(4)[None, :, None]   # g
                  + np.arange(64)[None, None, :]).reshape(-1)
        put("wo", _block(Wo.T[wo_idx][:, ch].astype(bf), 8))

        xf = np.zeros((128, 400), dtype=np.float32)
        xf[:, 0:8] = bq.reshape(8, 128).T
        xf[:, 8:10] = bk[ch].reshape(2, 128).T
        xf[:, 10:12] = bo[ch].reshape(2, 128).T
        xf[0, 16:272] = bv[ch]
        xf[0, 272:400] = 1.0
        in_maps.append({"xb": xb, "xf": xf})
    return in_maps


def assemble(results, S):
    out = np.empty((B, S, D), dtype=np.float32)
    for c in range(NCORES):
        b, g = divmod(c, 4)
        out[b, :, 256 * g:256 * g + 256] = results[c]["ofinal"].T
    return out


def kernel(**inputs):
    S = int(np.asarray(inputs["queries"]).shape[1])
    nc = _get_module(S)
    in_maps = host_shard(inputs, S)
    res = run_bass_kernel_spmd(nc, in_maps, core_ids=list(range(NCORES)))
    return assemble(res.results, S)
